# revision 5
# baseline (speedup 1.0000x reference)
"""Trainium2 Bass kernel: 7x7 valid cross-correlation + bias on a 4096x4096 f32 image.

Formulation: banded matmul on the TensorEngine.
  out[r, c] = sum_{di,dj} w[di,dj] * x[r+di, c+dj]
For an output row-strip of M=122 rows starting at r0, using K=128 input rows:
  out[r0+m, c] = sum_k A_dj[k, m] * x[r0+k, c+dj]   summed over dj=0..6
where A_dj[k, m] = w[k-m, dj] for 0 <= k-m < 7 (a banded [128, 122] matrix,
precomputed on host from the 49 kernel weights). The 7 dj-terms accumulate
into one PSUM bank via shifted column slices of the same SBUF rhs tile.

Sharding: output columns are split across the 8 cores (512 cols/core);
each core processes all 4090 output rows. Kernel + bias replicated.
"""

import numpy as np

H, W = 4096, 4096
KH, KW = 7, 7
OH, OW = H - KH + 1, W - KW + 1  # 4090, 4090
N_CORES = 8
CW = 512               # output columns per core
IW = CW + KW - 1       # input columns per core (518)
STRIP = 122            # output rows per strip (K = STRIP + KH - 1 = 128)
N_STRIPS = (OH + STRIP - 1) // STRIP  # 34 (last strip M=64, K=70)

_cache = {}

# Matmul input dtype: "float32" (exact, 4 cycles/row) or "float32r"
# (fp32 storage, fast-mode matmul at 1 cycle/row for N>=256).
MM_DTYPE = "float32r"


def _build_nc():
    import concourse.bacc as bacc
    import concourse.mybir as mybir
    from concourse.tile import TileContext

    dt = getattr(mybir.dt, MM_DTYPE)

    f32 = mybir.dt.float32
    nc = bacc.Bacc("TRN2", target_bir_lowering=False, debug=False)
    xs = nc.dram_tensor("xs", [H, IW], dt, kind="ExternalInput")
    bands = nc.dram_tensor("bands", [128, KW * STRIP], dt, kind="ExternalInput")
    biasv = nc.dram_tensor("biasv", [128, 1], f32, kind="ExternalInput")
    out = nc.dram_tensor("out", [OH, CW], f32, kind="ExternalOutput")

    with TileContext(nc) as tc:
        with (
            tc.tile_pool(name="const", bufs=1) as cpool,
            tc.tile_pool(name="rhs", bufs=3) as rpool,
            tc.tile_pool(name="obuf", bufs=3) as opool,
            tc.tile_pool(name="psum", bufs=4, space="PSUM") as ppool,
        ):
            band_t = cpool.tile([128, KW * STRIP], dt)
            nc.sync.dma_start(out=band_t[:, :], in_=bands[:, :])
            bias_t = cpool.tile([128, 1], f32)
            nc.sync.dma_start(out=bias_t[:, :], in_=biasv[:, :])

            for s in range(N_STRIPS):
                r0 = s * STRIP
                M = min(STRIP, OH - r0)
                K = min(128, H - r0)
                rhs_t = rpool.tile([128, IW], dt)
                nc.sync.dma_start(out=rhs_t[:K, :], in_=xs[r0 : r0 + K, :])
                ps = ppool.tile([128, CW], mybir.dt.float32)
                for dj in range(KW):
                    nc.tensor.matmul(
                        ps[:M, :],
                        band_t[:K, dj * STRIP : dj * STRIP + M],
                        rhs_t[:K, dj : dj + CW],
                        start=(dj == 0),
                        stop=(dj == KW - 1),
                    )
                ot = opool.tile([128, CW], f32)
                nc.vector.tensor_scalar_add(ot[:M, :], ps[:M, :], bias_t[:M, :1])
                nc.sync.dma_start(out=out[r0 : r0 + M, :], in_=ot[:M, :])

    nc.finalize()
    return nc


def _get_nc():
    if "nc" not in _cache:
        _cache["nc"] = _build_nc()
    return _cache["nc"]


def _build_bands(weight: np.ndarray) -> np.ndarray:
    """bands[k, dj*STRIP + m] = weight[k - m, dj] for 0 <= k-m < KH."""
    bands = np.zeros((128, KW * STRIP), np.float32)
    m = np.arange(STRIP)
    for dj in range(KW):
        for di in range(KH):
            bands[m + di, dj * STRIP + m] = weight[di, dj]
    return bands


def _prepare_in_maps(x, weight, bias):
    x = np.ascontiguousarray(x, np.float32)
    bands = _build_bands(np.asarray(weight, np.float32))
    bias_tile = np.full((128, 1), np.float32(np.asarray(bias).reshape(-1)[0]))

    in_maps = []
    for c in range(N_CORES):
        c0 = c * CW
        avail = min(IW, W - c0)
        if avail == IW:
            xs = x[:, c0 : c0 + IW]
        else:
            xs = np.zeros((H, IW), np.float32)
            xs[:, :avail] = x[:, c0 : c0 + avail]
        in_maps.append({"xs": xs, "bands": bands, "biasv": bias_tile})
    return in_maps


def _gather_out(per_core_outs) -> np.ndarray:
    out = np.empty((OH, OW), np.float32)
    for c in range(N_CORES):
        c0 = c * CW
        take = min(CW, OW - c0)
        out[:, c0 : c0 + take] = per_core_outs[c]["out"][:, :take]
    return out


def kernel(x: np.ndarray, weight: np.ndarray, bias: np.ndarray) -> np.ndarray:
    from concourse import bass_utils

    nc = _get_nc()
    in_maps = _prepare_in_maps(x, weight, bias)
    res = bass_utils.run_bass_kernel_spmd(nc, in_maps, list(range(N_CORES)))
    _cache["last_results"] = res
    return _gather_out(res.results)
